# revision 30
# baseline (speedup 1.0000x reference)
"""Trainium2 Bass kernel for a 2-layer tanh RNN.

B=64, T=512, I=H=1024, L=2.  Data-parallel over batch: 8 cores x 8 batch.
Per core:
  A1 (prologue): xw0 = X @ W_ih0^T + b0   (fp16 GEMM, fp32 accumulate)
  Stream: layer-0 recurrence, layer-1 input projection (from the layer-0
  hidden-state ring in SBUF, 16 steps per chunk), and layer-1 recurrence
  (lagged LAG steps) all interleaved so PE/DVE/ACT dependency stalls of
  one chain are filled by the others.

Everything on the matmul path is fp16 (values here are tame: h in
[-1,1], X ~ N(0,1), weights ~ N(0, 1/H); fp16's 10-bit mantissa keeps
the end-to-end error ~100x under bf16), accumulation is fp32 in PSUM.

Recurrence step: pre-activation accumulates in two PSUM banks (one per
j-half).  An accumulation group must be the only open group in its PSUM
bank, so each j-half is two sequential groups per bank:
  strip0 (psum rows 0:8):   I8-identity preload of xw_t + chunks kc 0..3
  strip1 (psum rows 32:40): contraction chunks kc 4..7
and the emission cross-pairs strip0@jh0 with strip1@jh1 (different banks
AND different PE col-strips -> they overlap on the array), then
strip0@jh1 with strip1@jh0.
Epilogue: add strips, tanh -> h_new [8, 1024] fp32; PE-transpose ->
hT ring [128, kc, RING, 8] fp16 = stationary operand of the next step's
matmuls and of the layer-1 projection chunks.
"""

import os
import sys

sys.path.insert(0, "/opt/trn_rl_repo")

import numpy as np
import ml_dtypes

import concourse.bass as bass
import concourse.bacc as bacc
import concourse.mybir as mybir
from concourse import tile
from concourse.bass_utils import run_bass_kernel_spmd

F32 = mybir.dt.float32
F16 = mybir.dt.float16

N_CORES = 8
B, T_FULL, I_DIM, H_DIM = 64, 512, 1024, 1024
BC = B // N_CORES  # 8 batch per core
KC = 8             # contraction chunks of 128
KH = 4             # chunks per strip
RING = 32          # hT ring slots
TB = 2             # xw prefetch block (steps)
LAG = 32           # layer-1 lag behind layer-0
TANH = mybir.ActivationFunctionType.Tanh


def _build(T, reps=None):
    """Build + compile the SPMD program (same on all cores).

    reps: if given, wrap the whole compute in a hardware For_i loop that
    repeats it `reps` times (used only for wall-clock timing)."""
    nc = bacc.Bacc("TRN2", target_bir_lowering=False, debug=False,
                   num_devices=N_CORES)
    NTOK = T * BC
    NTC = NTOK // 128  # token chunks

    # ---- external inputs -------------------------------------------------
    xt_d = nc.dram_tensor("xt", [KC, 128, NTOK], F16, kind="ExternalInput").ap()
    w0t_d = nc.dram_tensor("w0t", [KC, 128, H_DIM], F16, kind="ExternalInput").ap()
    w1t_d = nc.dram_tensor("w1t", [KC, 128, H_DIM], F16, kind="ExternalInput").ap()
    wh0t_d = nc.dram_tensor("wh0t", [KC, 128, H_DIM], F16, kind="ExternalInput").ap()
    wh1t_d = nc.dram_tensor("wh1t", [KC, 128, H_DIM], F16, kind="ExternalInput").ap()
    b0_d = nc.dram_tensor("b0", [1, H_DIM], F16, kind="ExternalInput").ap()
    b1_d = nc.dram_tensor("b1", [1, H_DIM], F16, kind="ExternalInput").ap()
    i8h_d = nc.dram_tensor("i8h", [8, 8], F16, kind="ExternalInput").ap()
    i8f_d = nc.dram_tensor("i8f", [8, 8], F32, kind="ExternalInput").ap()

    # ---- external outputs ------------------------------------------------
    ys_d = nc.dram_tensor("ys", [BC, T, H_DIM], F32, kind="ExternalOutput").ap()
    hlast_d = nc.dram_tensor("hlast", [2, BC, H_DIM], F32, kind="ExternalOutput").ap()

    with tile.TileContext(nc) as tc:
        with (
            tc.tile_pool(name="dram", bufs=1, space="DRAM") as dram,
            tc.tile_pool(name="misc", bufs=1) as misc,
        ):
            xw_d = [dram.tile([T, BC, H_DIM], F16, name=f"xwd{l}",
                              tag=f"xwd{l}") for l in range(2)]

            i8h = misc.tile([8, 8], F16)
            nc.sync.dma_start(i8h[:], i8h_d[:])
            i8f = misc.tile([8, 8], F32)
            nc.sync.dma_start(i8f[:], i8f_d[:])
            ones = misc.tile([1, 128], F16)
            nc.gpsimd.memset(ones[:], 1.0)
            b_sb = misc.tile([1, 2, H_DIM], F16)
            nc.sync.dma_start(b_sb[:, 0, :], b0_d[:])
            nc.sync.dma_start(b_sb[:, 1, :], b1_d[:])

            import contextlib
            rep_ctx = (tc.For_i(0, reps, 1) if reps
                       else contextlib.nullcontext())
            # ------------- all pools (one scope; ~180KB/partition) --------
            with (
                tc.tile_pool(name="a0_w", bufs=1) as wpool0,
                tc.tile_pool(name="a0_x", bufs=1) as xpool,
                tc.tile_pool(name="a0_st", bufs=3) as stpool0,
                tc.tile_pool(name="s_w", bufs=1) as wpool,
                tc.tile_pool(name="s_ht", bufs=1) as htpool,
                tc.tile_pool(name="s_xw", bufs=2) as xwpool,
                tc.tile_pool(name="s_hn", bufs=2) as hnpool,
                tc.tile_pool(name="s_ep", bufs=2) as eppool,
                tc.tile_pool(name="s_st", bufs=3) as stpool,
                tc.tile_pool(name="s_ps0", bufs=3, space="PSUM") as ps0pool,
                tc.tile_pool(name="s_ps1", bufs=3, space="PSUM") as ps1pool,
                tc.tile_pool(name="s_hp", bufs=2, space="PSUM") as hppool,
                rep_ctx,
            ):
                # ---------- prologue: layer-0 projection ------------------
                w0 = wpool0.tile([128, KC, H_DIM], F16)
                for ic in range(KC):
                    nc.sync.dma_start(w0[:, ic, :], w0t_d[ic, :, :])
                xt = xpool.tile([128, KC, NTOK], F16)
                for ic in range(KC):
                    nc.sync.dma_start(xt[:, ic, :], xt_d[ic, :, :])
                xw0_flat = xw_d[0][:].rearrange("t b h -> (t b) h")
                for tc_ in range(NTC):
                    for jh in range(2):
                        sl = slice(jh * 512, (jh + 1) * 512)
                        p = ps0pool.tile([128, 512], F32,
                                         name="pa1", tag="ps0")
                        nc.tensor.matmul(p[:], ones[:], b_sb[:, 0, sl],
                                         start=True, stop=False)
                        for ic in range(KC):
                            nc.tensor.matmul(
                                p[:], xt[:, ic, tc_ * 128:(tc_ + 1) * 128],
                                w0[:, ic, sl], start=False, stop=(ic == KC - 1))
                        st = stpool0.tile([128, 512], F16, tag="st0")
                        if jh == 0:
                            nc.vector.tensor_copy(st[:], p[:])
                        else:
                            nc.scalar.copy(st[:], p[:])
                        nc.sync.dma_start(
                            xw0_flat[tc_ * 128:(tc_ + 1) * 128, sl], st[:])

                # ---------- main stream -----------------------------------
                wh = [None, None]
                wh[0] = wpool.tile([128, KC, H_DIM], F16, name="wh0s",
                                   tag="wh0s")
                wh[1] = wpool.tile([128, KC, H_DIM], F16, name="wh1s",
                                   tag="wh1s")
                w1 = wpool.tile([128, KC, H_DIM], F16)
                for ic in range(KC):
                    nc.sync.dma_start(wh[0][:, ic, :], wh0t_d[ic, :, :])
                    nc.sync.dma_start(wh[1][:, ic, :], wh1t_d[ic, :, :])
                    nc.sync.dma_start(w1[:, ic, :], w1t_d[ic, :, :])
                ht = [None, None]
                ht[0] = htpool.tile([128, KC, RING, BC], F16, name="ht0s",
                                    tag="ht0s")
                ht[1] = htpool.tile([128, KC, RING, BC], F16, name="ht1s",
                                    tag="ht1s")
                xw_sb = [None, None]
                xw1_flat = xw_d[1][:].rearrange("t b h -> (t b) h")
                pspool = [ps0pool, ps1pool]

                def do_step(l, t):
                    if t % TB == 0:
                        xw_sb[l] = xwpool.tile([BC, TB, H_DIM], F16,
                                               name=f"xwsb{l}", tag=f"xwsb{l}")
                        nc.sync.dma_start(
                            xw_sb[l][:],
                            xw_d[l][t:t + TB, :, :].rearrange("t b h -> b t h"))
                    prev = (t - 1) % RING
                    slot = t % RING
                    hp = hppool.tile([128, 64], F16, tag="hp")
                    sl0, sl1 = slice(0, 512), slice(512, 1024)
                    xw0_r = xw_sb[l][:, t % TB, sl0]
                    xw1_r = xw_sb[l][:, t % TB, sl1]
                    p0 = pspool[l].tile([64, 512], F32, name=f"pA{l}",
                                        tag=f"ps{l}")
                    p1 = pspool[l].tile([64, 512], F32, name=f"pB{l}",
                                        tag=f"ps{l}")
                    s_t = [None, None]
                    if t == 0:
                        nc.tensor.matmul(p0[0:8, :], i8h[:], xw0_r,
                                         start=True, stop=True)
                        nc.tensor.matmul(p1[0:8, :], i8h[:], xw1_r,
                                         start=True, stop=True)
                        for jh, p in ((0, p0), (1, p1)):
                            s = eppool.tile([8, 512], F16, name="s", tag="s")
                            nc.vector.tensor_copy(s[:], p[0:8, :])
                            s_t[jh] = s
                    else:
                        # phase 1: strip0@jh0 (with xw preload) || strip1@jh1
                        # -- different PSUM banks and PE col-strips, so the
                        # two open accumulation groups overlap on the array.
                        nc.tensor.matmul(p0[0:8, :], i8h[:], xw0_r,
                                         start=True, stop=False)
                        for i in range(KH):
                            nc.tensor.matmul(
                                p0[0:8, :], ht[l][:, i, prev, :],
                                wh[l][:, i, sl0],
                                start=False, stop=(i == KH - 1))
                            nc.tensor.matmul(
                                p1[32:40, :], ht[l][:, i + KH, prev, :],
                                wh[l][:, i + KH, sl1],
                                start=(i == 0), stop=(i == KH - 1))
                        # phase 2: strip0@jh1 (xw folded on DVE) || strip1@jh0
                        for i in range(KH):
                            nc.tensor.matmul(
                                p1[0:8, :], ht[l][:, i, prev, :],
                                wh[l][:, i, sl1],
                                start=(i == 0), stop=(i == KH - 1))
                            nc.tensor.matmul(
                                p0[32:40, :], ht[l][:, i + KH, prev, :],
                                wh[l][:, i + KH, sl0],
                                start=(i == 0), stop=(i == KH - 1))
                        # jh0 epilogue: ACT copy + DVE add (xw already in PSUM)
                        s0 = eppool.tile([8, 512], F16, name="s0", tag="s")
                        s32 = eppool.tile([8, 512], F32, name="s32", tag="s32")
                        nc.scalar.copy(s32[:], p0[32:40, :])
                        nc.vector.tensor_add(s0[:], p0[0:8, :], s32[:])
                        # jh1 epilogue: fold xw via DVE, then add strips
                        s1 = eppool.tile([8, 512], F16, name="s1", tag="s2")
                        s32b = eppool.tile([8, 512], F32, name="s32b",
                                           tag="s32b")
                        nc.vector.tensor_add(s32b[:], p1[32:40, :], xw1_r)
                        nc.vector.tensor_add(s1[:], p1[0:8, :], s32b[:])
                        s_t = [s0, s1]
                    # transposes read the fp16 pre-activations (FWL-fast),
                    # tanh is applied post-transpose on [128, 64].
                    for kc in range(KC):
                        nc.tensor.transpose(
                            hp[:, kc * 8:(kc + 1) * 8],
                            s_t[kc // KH][:, (kc % KH) * 128:
                                          (kc % KH + 1) * 128], i8h[:])
                    hview = hp[:].rearrange("p (kc b) -> p kc b", b=BC)
                    nc.scalar.activation(ht[l][:, :, slot, :], hview, TANH)
                    if l == 1 or t == T - 1:
                        hn = hnpool.tile([BC, H_DIM], F32, name=f"hn{l}",
                                         tag=f"hn{l}")
                        nc.scalar.activation(hn[:, sl0], s_t[0][:], TANH)
                        nc.scalar.activation(hn[:, sl1], s_t[1][:], TANH)
                        if l == 1:
                            nc.sync.dma_start(ys_d[:, t, :], hn[:])
                        if t == T - 1:
                            nc.sync.dma_start(hlast_d[l, :, :], hn[:])

                def do_a2_chunk(k):
                    # project layer-0 hidden states for steps 16k..16k+16
                    s0 = (k * 16) % RING
                    for jh in range(2):
                        sl = slice(jh * 512, (jh + 1) * 512)
                        p = ps0pool.tile([128, 512], F32, name="pa2",
                                         tag="ps0")
                        nc.tensor.matmul(p[:], ones[:], b_sb[:, 1, sl],
                                         start=True, stop=False)
                        for ic in range(KC):
                            nc.tensor.matmul(
                                p[:], ht[0][:, ic, s0:s0 + 16, :],
                                w1[:, ic, sl], start=False,
                                stop=(ic == KC - 1))
                        st = stpool.tile([128, 512], F16, tag="st")
                        if jh == 0:
                            nc.vector.tensor_copy(st[:], p[:])
                        else:
                            nc.scalar.copy(st[:], p[:])
                        nc.sync.dma_start(
                            xw1_flat[k * 128:(k + 1) * 128, sl], st[:])

                for u in range(T + LAG):
                    if u < T:
                        do_step(0, u)
                        if u % 16 == 15:
                            do_a2_chunk(u // 16)
                    if u >= LAG:
                        do_step(1, u - LAG)

    nc.compile()
    return nc


_NC_CACHE = {}


def _get_nc(T):
    if T not in _NC_CACHE:
        _NC_CACHE[T] = _build(T)
    return _NC_CACHE[T]


def _prep_core_inputs(X_c, W, T):
    """X_c: [BC, T, I] fp32.  W: dict of shared weights.  Returns in_map."""
    x_tok = np.transpose(X_c, (1, 0, 2)).reshape(T * BC, I_DIM)
    xt = np.ascontiguousarray(x_tok.T).reshape(KC, 128, T * BC).astype(np.float16)
    return {"xt": xt, **W,
            "i8h": np.eye(8, dtype=np.float16),
            "i8f": np.eye(8, dtype=np.float32)}


def _prep_weights(W_ih0, b_ih0, W_hh0, b_hh0, W_ih1, b_ih1, W_hh1, b_hh1):
    def rt(w):  # [H, K] -> [K_chunks, 128, H] fp16
        return np.ascontiguousarray(w.T).reshape(KC, 128, H_DIM).astype(np.float16)
    return {
        "w0t": rt(W_ih0), "w1t": rt(W_ih1),
        "wh0t": rt(W_hh0), "wh1t": rt(W_hh1),
        "b0": (b_ih0 + b_hh0).reshape(1, H_DIM).astype(np.float16),
        "b1": (b_ih1 + b_hh1).reshape(1, H_DIM).astype(np.float16),
    }


def run(X, W_ih0, b_ih0, W_hh0, b_hh0, W_ih1, b_ih1, W_hh1, b_hh1,
        T=T_FULL, trace=False):
    X = np.asarray(X, dtype=np.float32)
    nc = _get_nc(T)
    W = _prep_weights(np.asarray(W_ih0), np.asarray(b_ih0),
                      np.asarray(W_hh0), np.asarray(b_hh0),
                      np.asarray(W_ih1), np.asarray(b_ih1),
                      np.asarray(W_hh1), np.asarray(b_hh1))
    in_maps = [_prep_core_inputs(X[c * BC:(c + 1) * BC, :T], W, T)
               for c in range(N_CORES)]
    try:
        res = run_bass_kernel_spmd(nc, in_maps, core_ids=list(range(N_CORES)),
                                   trace=trace)
    except Exception:
        # transient device wedges (NRT_EXEC_UNIT_UNRECOVERABLE) clear on a
        # fresh dispatch; retry once
        res = run_bass_kernel_spmd(nc, in_maps, core_ids=list(range(N_CORES)),
                                   trace=trace)
    ys = np.concatenate([res.results[c]["ys"] for c in range(N_CORES)], axis=0)
    hl = np.concatenate([res.results[c]["hlast"] for c in range(N_CORES)],
                        axis=1)
    return (ys, hl), res


def kernel(X, W_ih0, b_ih0, W_hh0, b_hh0, W_ih1, b_ih1, W_hh1, b_hh1):
    out, _ = run(X, W_ih0, b_ih0, W_hh0, b_hh0, W_ih1, b_ih1, W_hh1, b_hh1)
    return out


def _build_noop(T):
    """Same external I/O as _build, but ~zero device work.  Used only by
    test.py to subtract host/transfer/dispatch overhead from wall time."""
    nc = bacc.Bacc("TRN2", target_bir_lowering=False, debug=False,
                   num_devices=N_CORES)
    NTOK = T * BC
    for nm, shp, dt_ in [("xt", [KC, 128, NTOK], F16),
                         ("w0t", [KC, 128, H_DIM], F16),
                         ("w1t", [KC, 128, H_DIM], F16),
                         ("wh0t", [KC, 128, H_DIM], F16),
                         ("wh1t", [KC, 128, H_DIM], F16),
                         ("b0", [1, H_DIM], F16), ("b1", [1, H_DIM], F16),
                         ("i8h", [8, 8], F16), ("i8f", [8, 8], F32)]:
        nc.dram_tensor(nm, shp, dt_, kind="ExternalInput")
    ys_d = nc.dram_tensor("ys", [BC, T, H_DIM], F32, kind="ExternalOutput").ap()
    hlast_d = nc.dram_tensor("hlast", [2, BC, H_DIM], F32,
                             kind="ExternalOutput").ap()
    with tile.TileContext(nc) as tc:
        with tc.tile_pool(name="z", bufs=1) as zp:
            z = zp.tile([BC, H_DIM], F32)
            nc.gpsimd.memset(z[:], 0.0)
            nc.sync.dma_start(ys_d[:, 0, :], z[:])
            nc.sync.dma_start(hlast_d[0, :, :], z[:])
            nc.sync.dma_start(hlast_d[1, :, :], z[:])
    nc.compile()
    return nc


def bench(X, W_ih0, b_ih0, W_hh0, b_hh0, W_ih1, b_ih1, W_hh1, b_hh1,
          T=T_FULL, reps=3):
    """Return (real_walls, noop_walls) lists of per-call seconds (cached)."""
    import time as _time
    X = np.asarray(X, dtype=np.float32)
    W = _prep_weights(np.asarray(W_ih0), np.asarray(b_ih0),
                      np.asarray(W_hh0), np.asarray(b_hh0),
                      np.asarray(W_ih1), np.asarray(b_ih1),
                      np.asarray(W_hh1), np.asarray(b_hh1))
    in_maps = [_prep_core_inputs(X[c * BC:(c + 1) * BC, :T], W, T)
               for c in range(N_CORES)]
    nc_real = _get_nc(T)
    if ("noop", T) not in _NC_CACHE:
        _NC_CACHE[("noop", T)] = _build_noop(T)
    nc_noop = _NC_CACHE[("noop", T)]
    walls = {"real": [], "noop": []}
    for kind, nc_ in [("noop", nc_noop), ("real", nc_real)]:
        run_bass_kernel_spmd(nc_, in_maps, core_ids=list(range(N_CORES)))
        for _ in range(reps):
            t0 = _time.perf_counter()
            run_bass_kernel_spmd(nc_, in_maps, core_ids=list(range(N_CORES)))
            walls[kind].append(_time.perf_counter() - t0)
    return walls["real"], walls["noop"]
